# revision 78
# baseline (speedup 1.0000x reference)
"""Trainium2 Bass kernel for nn_Net_66975720014255 (gnn_message_passing).

Sharding: data-parallel over batch B=32 across 8 NeuronCores (4 batches per
core); adjacency and all weights replicated. No collectives.

v2 design (per core, C=40, T=12, N=800, R=11):
  - everything bf16 (PSUM accumulation f32); output shipped bf16, host upcasts
  - weight/window matrices column-padded so every matmul has M=128 (FWL)
  - node dim K padded 1600 -> 1664 = 13x128 uniform K-tiles
  - adjacency computed ON DEVICE from nv1/nv2 (0.26MB DMA instead of 5.1MB,
    and the K=40 matmuls warm the PE during the input-DMA ramp)
  - optional hop1 in fp8-e4m3 DoubleRow (2x K-rate): adj upper half + h1t are
    cast to fp8 on device; numerically validated (rel err ~4e-3 total)
  - warm-up matmuls at t=0 keep HAM at full clock through the DMA ramp
  - few, large DMAs on the idle sync/gpsimd queues; last batch's residual
    matmuls hoisted before the hops so the tail is only skip + out-DMA drain
"""

import sys

if '/opt/trn_rl_repo' not in sys.path:
    sys.path.insert(0, '/opt/trn_rl_repo')

import numpy as np
import ml_dtypes

import concourse.bass as bass  # noqa: F401
import concourse.tile as tile
from concourse import bacc, mybir
from concourse.bass_utils import run_bass_kernel_spmd

# ----- problem constants (hardcoded per contract) -----
B, C, T, N = 32, 40, 12, 800
R = T - 1                    # 11
N2 = 2 * N                   # 1600
NCORES = 8
BL = B // NCORES             # 4 local batches per core
BN_SCALE = float(1.0 / np.sqrt(1.0 + 1e-5))

Q = T * C                    # 480 rows (t,c) per batch
RQ = R * C                   # 440 rows (r,c) per batch
NKT = 13                     # node K-tiles: 13*128 = 1664 (1600 padded)
N2P = NKT * 128              # 1664
MB = [(0, 120), (120, 120), (240, 120), (360, 80)]   # (r,c) row blocks (real)
CH800 = [(0, 400), (400, 400)]
CH1600 = [(0, 400), (400, 400), (800, 400), (1200, 400)]

F32 = mybir.dt.float32
BF16 = mybir.dt.bfloat16
F8E4 = mybir.dt.float8e4

HOP1_FP8 = True              # hop1 via fp8 DoubleRow (2x K-rate)
ADJ_FP8 = True               # single fp8 adjacency: hop0 rhs fp8 (lhsT bf16)
WARMUP_MM = 24

_bf = ml_dtypes.bfloat16
_f8 = ml_dtypes.float8_e4m3


# ---------------------------------------------------------------------------
# host-side preparation (pure numpy)
# ---------------------------------------------------------------------------

def _pad512(Wb):
    """[rows, 440-or-480] -> [rows, 512]: each 120-col block padded to 128
    (440-col input: 4th block is 80 wide)."""
    rows, cols = Wb.shape
    out = np.zeros((rows, 512), np.float32)
    for m in range(4):
        w = min(120, cols - m * 120)
        out[:, m * 128:m * 128 + w] = Wb[:, m * 120:m * 120 + w]
    return out


def _ktile4(Wp, dt=None, part=120):
    """[rows<=480, 512] -> [part, 4, 512]: K-tiles of 120 real rows,
    zero-padded to `part` partitions."""
    full = np.zeros((480, 512), np.float32)
    full[:Wp.shape[0]] = Wp
    out = np.zeros((part, 4, 512), np.float32)
    out[:120] = full.reshape(4, 120, 512).transpose(1, 0, 2)
    return np.ascontiguousarray(out.astype(dt or _bf))


def _prep_weights(inp):
    f32 = np.float32
    nv1 = np.asarray(inp['nv1'], f32)          # (1600, 40)
    nv2 = np.asarray(inp['nv2'], f32)          # (40, 1600)
    nv1t = np.zeros((40, N2P), f32)
    nv1t[:, :N2] = nv1.T
    nv1t = np.ascontiguousarray(nv1t.astype(_bf))
    nv2b = np.ascontiguousarray(nv2.astype(_bf))

    def wbig(W):
        Wb = np.zeros((Q, RQ), f32)
        W0, W1 = np.asarray(W[:, :, 0], f32), np.asarray(W[:, :, 1], f32)
        for r in range(R):
            Wb[r * C:(r + 1) * C, r * C:(r + 1) * C] = W0.T
            Wb[(r + 1) * C:(r + 2) * C, r * C:(r + 1) * C] = W1.T
        return Wb

    wbig2 = np.ascontiguousarray(
        np.stack([_ktile4(_pad512(wbig(np.asarray(inp['W_f']))), _f8, 128),
                  _ktile4(_pad512(wbig(np.asarray(inp['W_g']))), _f8, 128)]
                 ).transpose(1, 0, 2, 3))          # [128, 2, 4, 512] fp8

    def blkdiag3(A, cols=120):                 # A (40, 40) -> [120, cols]
        M = np.zeros((120, cols), f32)
        for j in range(3):
            M[j * C:(j + 1) * C, j * C:(j + 1) * C] = A
        return M

    wmix1 = np.ascontiguousarray(
        blkdiag3(np.asarray(inp['W_gcn'][0], f32).T, 128).astype(_bf))
    wmix2 = np.ascontiguousarray(
        blkdiag3(np.asarray(inp['W_gcn'][1], f32).T, 128).astype(_bf))

    eye = np.eye(C, dtype=f32)
    Ws = np.asarray(inp['W_skip'], f32) * BN_SCALE            # (12, 11)
    wskip = np.zeros((RQ, 480), f32)
    for s in range(12):
        for r in range(R):
            wskip[r * C:(r + 1) * C, s * C:(s + 1) * C] = Ws[s, r] * eye
    wskip = _ktile4(_pad512(wskip))

    wres = np.zeros((Q, RQ), f32)
    Wr = np.asarray(inp['W_res'], f32) * BN_SCALE             # (11, 12)
    for t in range(T):
        for r in range(R):
            wres[t * C:(t + 1) * C, r * C:(r + 1) * C] = Wr[r, t] * eye
    wres = _ktile4(_pad512(wres), _f8, 128)

    bias_f = np.ascontiguousarray(np.tile(np.asarray(inp['b_f'], f32), 3)[:, None])
    bias_g = np.ascontiguousarray(np.tile(np.asarray(inp['b_g'], f32), 3)[:, None])

    bs = np.asarray(inp['b_skip'], f32) * BN_SCALE
    bskip_tile = np.zeros((120, 4, 1), f32)
    for sm in range(4):
        for p in range(120):
            bskip_tile[p, sm, 0] = bs[sm * 3 + p // C]
    bres = np.asarray(inp['b_res'], f32) * BN_SCALE
    bres_tile = np.zeros((120, 1), f32)
    for p in range(120):
        r = p // C
        bres_tile[p, 0] = bres[r] if r < R else 0.0

    return dict(nv1t=nv1t, nv2=nv2b, wbig=wbig2, wmix1=wmix1, wmix2=wmix2,
                wskip=wskip, wres=wres, bias_f=bias_f, bias_g=bias_g,
                bskip_tile=np.ascontiguousarray(bskip_tile),
                has_bskip=bool(np.any(bs)),
                bres_tile=bres_tile, has_bres=bool(np.any(bres)))


def _prep_data(inp):
    f32 = np.float32
    x = np.asarray(inp['x'], f32) + np.asarray(inp['t_emb'], f32) \
        + np.asarray(inp['s_emb'], f32)                        # (B,C,T,N)
    xp = np.ascontiguousarray(x.transpose(0, 2, 1, 3)).reshape(B, Q, N)
    # xp tiles (fp8 for tconv/res DoubleRow): [B, 128, 4, 800]
    xp_t = np.zeros((B, 128, 4, N), np.float32)
    xp_t[:, :120] = xp.reshape(B, 4, 120, N).transpose(0, 2, 1, 3)
    xp_t = np.ascontiguousarray(xp_t.astype(_f8))
    # windowed transpose, rows k in [0,1600): k<800 -> x'T[:, :440];
    # k>=800 -> x'T[:, 40:480]; cols padded per 120->128 block, rows ->1664
    xpt = np.ascontiguousarray(x.transpose(0, 3, 2, 1)).reshape(B, N, Q)
    wxt = np.zeros((B, N2P, 512), f32)
    for m in range(4):
        w = 120 if m < 3 else 80
        lo, hi = m * 120, m * 120 + w
        wxt[:, :N, m * 128:m * 128 + w] = xpt[:, :, lo:hi]
        wxt[:, N:N2, m * 128:m * 128 + w] = xpt[:, :, C + lo:C + hi]
    wxt = np.ascontiguousarray(
        wxt.reshape(B, NKT, 128, 512).transpose(0, 2, 1, 3).astype(_bf))
    xp_cores = [np.ascontiguousarray(xp_t[i * BL:(i + 1) * BL])
                for i in range(NCORES)]
    wxt_cores = [np.ascontiguousarray(wxt[i * BL:(i + 1) * BL])
                 for i in range(NCORES)]
    return xp_cores, wxt_cores


# ---------------------------------------------------------------------------
# device program
# ---------------------------------------------------------------------------

def _build_program(has_bres, has_bskip):
    nc = bacc.Bacc("TRN2", target_bir_lowering=False, debug=False,
                   enable_asserts=False, num_devices=NCORES)

    xp_d = nc.dram_tensor("xp", [BL, 128, 4, N], F8E4, kind="ExternalInput").ap()
    wxt_d = nc.dram_tensor("wxt", [BL, 128, NKT, 512], BF16,
                           kind="ExternalInput").ap()
    nv1t_d = nc.dram_tensor("nv1t", [40, N2P], BF16, kind="ExternalInput").ap()
    nv2_d = nc.dram_tensor("nv2", [40, N2], BF16, kind="ExternalInput").ap()
    wbig_d = nc.dram_tensor("wbig", [128, 2, 4, 512], F8E4,
                            kind="ExternalInput").ap()
    wmix1_d = nc.dram_tensor("wmix1", [120, 128], BF16, kind="ExternalInput").ap()
    wmix2_d = nc.dram_tensor("wmix2", [120, 128], BF16, kind="ExternalInput").ap()
    wskip_d = nc.dram_tensor("wskip", [120, 4, 512], BF16,
                             kind="ExternalInput").ap()
    wres_d = nc.dram_tensor("wres", [128, 4, 512], F8E4,
                            kind="ExternalInput").ap()
    biasf_d = nc.dram_tensor("bias_f", [120, 1], F32, kind="ExternalInput").ap()
    biasg_d = nc.dram_tensor("bias_g", [120, 1], F32, kind="ExternalInput").ap()
    bskip_d = nc.dram_tensor("bskip", [120, 4, 1], F32, kind="ExternalInput").ap()
    bres_d = nc.dram_tensor("bres", [120, 1], F32, kind="ExternalInput").ap()
    # out[b, 0, p, m, n] = final rows (r,c)=m*120+p (m=3: p<80 real)
    # out[b, 1, p, m, n] = skip rows (s,c)=m*120+p
    out_d = nc.dram_tensor("out", [BL, 2, 120, 4, N], BF16,
                           kind="ExternalOutput").ap()

    with tile.TileContext(nc) as tc:
        _emit(nc, tc, xp_d, wxt_d, nv1t_d, nv2_d, wbig_d, wmix1_d, wmix2_d,
              wskip_d, wres_d, biasf_d, biasg_d, bskip_d, bres_d, out_d,
              has_bres, has_bskip)
    nc.compile()
    return nc


def _emit(nc, tc, xp_d, wxt_d, nv1t_d, nv2_d, wbig_d, wmix1_d, wmix2_d,
          wskip_d, wres_d, biasf_d, biasg_d, bskip_d, bres_d, out_d,
          has_bres, has_bskip):
    from contextlib import ExitStack
    AF = mybir.ActivationFunctionType
    ALU = mybir.AluOpType
    DR = mybir.MatmulPerfMode.DoubleRow
    ctx = ExitStack()
    with ctx:
        const = ctx.enter_context(tc.tile_pool(name="const", bufs=1))
        xp_p = ctx.enter_context(tc.tile_pool(name="xp", bufs=2))
        xpt_p = ctx.enter_context(tc.tile_pool(name="xpt", bufs=2))
        dres_p = ctx.enter_context(tc.tile_pool(name="dres", bufs=1))
        hop0_p = ctx.enter_context(tc.tile_pool(name="hop0sb", bufs=1))
        h1t_p = ctx.enter_context(tc.tile_pool(name="h1t", bufs=1))
        h2_p = ctx.enter_context(tc.tile_pool(name="h2sb", bufs=1))
        oraw_p = ctx.enter_context(tc.tile_pool(name="oraw", bufs=1))
        tmp_p = ctx.enter_context(tc.tile_pool(name="tmp", bufs=2))
        fin_p = ctx.enter_context(tc.tile_pool(name="fin", bufs=2))
        psA = ctx.enter_context(tc.tile_pool(name="psA", bufs=6, space="PSUM"))
        psB = ctx.enter_context(tc.tile_pool(name="psB", bufs=2, space="PSUM"))

        # ---- warm-up: PE busy from t=0 so HAM reaches 2.4GHz early ----
        dummy = const.tile([128, 400], BF16, name="dummy")
        nc.vector.memset(dummy[:], 0.125)
        def warm_mm():
            ps = psB.tile([128, 400], F32, name="warm", tag="psB")
            nc.tensor.matmul(ps[:, :], dummy[:, 0:128], dummy[:, :],
                             start=True, stop=True)

        for i in range(WARMUP_MM):
            warm_mm()

        # ---- DMA: nv first (adj compute), xp+wbig (tconv), wxt per batch ----
        # nv first on two queues: adj compute is the PE's first real work and
        # the DMA rings take ~5us to produce their first bytes
        nv1t_sb = const.tile([40, N2P], BF16, name="nv1t")
        nc.sync.dma_start(nv1t_sb[:], nv1t_d[:])
        nv2_sb = const.tile([40, N2], BF16, name="nv2")
        nc.scalar.dma_start(nv2_sb[:], nv2_d[:])
        biasf_sb = const.tile([120, 1], F32, name="biasf")
        nc.sync.dma_start(biasf_sb[:], biasf_d[:])
        biasg_sb = const.tile([120, 1], F32, name="biasg")
        nc.sync.dma_start(biasg_sb[:], biasg_d[:])

        def load_xp(b):
            t = xp_p.tile([128, 4, N], F8E4, name="xp", tag="xp", bufs=2)
            nc.sync.dma_start(t[:], xp_d[b])
            return t

        xp0 = load_xp(0)
        # wbig on the scalar queue: issues at t=0 in parallel with xp0 (sync)
        # and nv/wxt (gpsimd) so tconv's inputs all land ~7us earlier
        wbig_sb = const.tile([128, 2, 4, 512], F8E4, name="wbig")
        nc.scalar.dma_start(wbig_sb[:], wbig_d[:])

        def load_wxt(b):
            t = xpt_p.tile([128, NKT, 512], BF16, name="wxt", tag="wxt", bufs=2)
            nc.gpsimd.dma_start(t[:], wxt_d[b])
            return t

        wxt0 = load_wxt(0)

        # remaining weights (needed from mix1 / epilogue onward)
        wmix1_sb = const.tile([120, 128], BF16, name="wmix1")
        nc.sync.dma_start(wmix1_sb[:], wmix1_d[:])
        wmix2_sb = const.tile([120, 128], BF16, name="wmix2")
        nc.sync.dma_start(wmix2_sb[:], wmix2_d[:])
        wres_sb = const.tile([128, 4, 512], F8E4, name="wres")
        nc.sync.dma_start(wres_sb[:], wres_d[:])
        wskip_sb = const.tile([120, 4, 512], BF16, name="wskip")
        nc.sync.dma_start(wskip_sb[:], wskip_d[:])
        bres_sb = const.tile([120, 1], F32, name="bres_t")
        nc.sync.dma_start(bres_sb[:], bres_d[:])
        if has_bskip:
            bskip_sb = const.tile([120, 4, 1], F32, name="bskip_t")
            nc.sync.dma_start(bskip_sb[:], bskip_d[:])

        # batches 0/1 prefetched here; 2/3 loaded inside the loop so the
        # tag-based buffer reuse sees its prior readers (WAR ordering)
        xp_sb_all = [xp0, load_xp(1), None, None]
        wxt_sb_all = [wxt0, load_wxt(1), None, None]

        # ---- adjacency on device: adj = relu(nv1 @ nv2) ----
        if ADJ_FP8:
            # single fp8 copy in DoubleRow-paired layout [128, ksub=2, 1600]:
            # hop0 reads it as a plain (mixed-dtype) rhs, hop1 as DR pairs
            adj8p = [const.tile([128, 2, N2], F8E4, name=f"adj8p{p}")
                     for p in range(7)]
            nc.vector.memset(adj8p[6][:, 1, :], 0.0)

            def adj_rhs(kt, co, cs):
                return adj8p[kt // 2][:, kt % 2, co:co + cs]

            def adj1_rhs(p, co, cs):
                return adj8p[p][:, :, 800 + co:800 + co + cs]
        else:
            adj_sb = [const.tile([128, N2], BF16, name=f"adj{i}")
                      for i in range(NKT)]

            def adj_rhs(kt, co, cs):
                return adj_sb[kt][:, co:co + cs]

            if HOP1_FP8:
                # paired fp8 copy of adj[:, 800:1600]: [128, ksub=2, 800] x 7
                adj8_sb = [const.tile([128, 2, N], F8E4, name=f"adj8_{p}")
                           for p in range(7)]
                nc.vector.memset(adj8_sb[6][:, 1, :], 0.0)

                def adj1_rhs(p, co, cs):
                    return adj8_sb[p][:, :, co:co + cs]   # k-tile 13 = zero pad
        def emit_adj(chunks):
            for (co, cs) in chunks:
                for i in range(NKT):
                    ps = psA.tile([128, 400], F32, name="adj_ps", tag="psA")
                    nc.tensor.matmul(ps[:, :], nv1t_sb[:, i * 128:(i + 1) * 128],
                                     nv2_sb[:, co:co + cs], start=True, stop=True)
                    # split the casts over ACT/DVE so neither queue alone
                    # paces the PSUM-bank recycling
                    eng = nc.scalar if i % 2 == 0 else nc.vector
                    if ADJ_FP8:
                        dst = adj8p[i // 2][:, i % 2, co:co + cs]
                    else:
                        dst = adj_sb[i][:, co:co + cs]
                    if eng is nc.scalar:
                        eng.activation(dst, ps[:, :], AF.Relu)
                    else:
                        eng.tensor_relu(dst, ps[:, :])
                    if not ADJ_FP8 and HOP1_FP8 and co >= 800:
                        eng2 = nc.vector if i % 2 == 0 else nc.scalar
                        if eng2 is nc.scalar:
                            eng2.activation(
                                adj8_sb[i // 2][:, i % 2,
                                                co - 800:co - 800 + cs],
                                ps[:, :], AF.Relu)
                        else:
                            eng2.tensor_relu(
                                adj8_sb[i // 2][:, i % 2,
                                                co - 800:co - 800 + cs],
                                ps[:, :])

        def tconv_b(b, xp_sb, dres_sb=None):
            if dres_sb is None:
                dres_sb = [dres_p.tile([120, N], BF16, name=f"dres{m}",
                                       tag=f"dres{m}", bufs=1) for m in range(4)]
            for m, (mo, ms) in enumerate(MB):
                gate_sb = {}
                for g, bias_sb in ((0, biasf_sb), (1, biasg_sb)):
                    gt = tmp_p.tile([120, N], BF16, name="gate",
                                    tag=f"gate{g}", bufs=2)
                    gate_sb[g] = gt
                    for (co, cs) in CH800:
                        ps = psA.tile([128, 400], F32, name="tc_ps", tag="psA")
                        if m < 3:
                            nc.tensor.matmul(
                                ps[:, :],
                                wbig_sb[:, g, m:m + 2, m * 128:(m + 1) * 128],
                                xp_sb[:, m:m + 2, co:co + cs],
                                start=True, stop=True, perf_mode=DR)
                        else:
                            nc.tensor.matmul(
                                ps[:, :],
                                wbig_sb[:, g, 3, m * 128:(m + 1) * 128],
                                xp_sb[:, 3, co:co + cs],
                                start=True, stop=True)
                        nc.scalar.activation(
                            gt[:, co:co + cs], ps[0:120, :],
                            AF.Tanh if g == 0 else AF.Sigmoid,
                            bias=bias_sb[:, :])
                nc.vector.tensor_mul(dres_sb[m][:, :],
                                     gate_sb[0][:, :], gate_sb[1][:, :])
            return dres_sb

        def res_mm(xp_sb, m, co, cs):
            ps = psA.tile([128, 400], F32, name="rs_ps", tag="psA")
            for j in range(2):
                nc.tensor.matmul(ps[:, :],
                                 wres_sb[:, 2 * j:2 * j + 2,
                                         m * 128:(m + 1) * 128],
                                 xp_sb[:, 2 * j:2 * j + 2, co:co + cs],
                                 start=(j == 0), stop=(j == 1), perf_mode=DR)
            return ps

        def hops_b(b, wxt_sb, dres_sb, oraw_sb, tail_hook=None):
            # hop0 + mix1 -> h1t ; hop1 + mix2 (+dres add) -> oraw
            h0_tiles = [hop0_p.tile([128, N2P], BF16, name="h0", tag=f"h0_{m}",
                                    bufs=1) for m in range(4)]
            # batch 0: ch-outer (each chunk only needs 13 adj casts, so hop0
            # starts before the full cast sweep). later batches: m-outer, so
            # each h0 tile's casts finish early and mix1's relus never wait.
            if b == 0:
                order = [(co, cs, m) for (co, cs) in CH1600 for m in range(4)]
            else:
                order = [(co, cs, m) for m in range(4) for (co, cs) in CH1600]
            for (co, cs, m) in order:
                ps = psA.tile([128, 400], F32, name="h0_ps", tag="psA")
                for kt in range(NKT):
                    nc.tensor.matmul(
                        ps[:, :],
                        wxt_sb[:, kt, m * 128:(m + 1) * 128],
                        adj_rhs(kt, co, cs),
                        start=(kt == 0), stop=(kt == NKT - 1))
                nc.vector.tensor_copy(h0_tiles[m][:, co:co + cs], ps[:, :])
            for m in range(4):
                nc.vector.memset(h0_tiles[m][:, N2:N2P], 0.0)
            if HOP1_FP8:
                # one tile [128, 14 ksub, 512]: slices [:, 2p:2p+2, mslice]
                # feed hop1's DoubleRow pairs directly
                h1t_all = h1t_p.tile([128, 14, 512], F8E4, name="h1t",
                                     tag="h1t", bufs=1)
                h1t = [h1t_all[:, 2 * p:2 * p + 2, :] for p in range(7)]
                nc.vector.memset(h1t_all[:, 13, :], 0.0)
            else:
                h1t_all = h1t_p.tile([128, NKT, 512], BF16, name="h1t",
                                     tag="h1t", bufs=1)
                h1t = [h1t_all[:, kt, :] for kt in range(NKT)]

            for m in range(4):
                h0 = h0_tiles[m]
                # 4 mix1 matmuls per PSUM bank, ONE wide relu per group:
                # amortizes the DVE per-op overhead so it outruns the PE
                for gi, i0 in enumerate(range(0, NKT, 4)):
                    g = min(4, NKT - i0)
                    bp = psB.tile([128, 4, 128], F32, name="b1_ps", tag="psB")
                    for j in range(g):
                        i = i0 + j
                        nc.tensor.matmul(bp[:, j, :],
                                         h0[0:120, i * 128:(i + 1) * 128],
                                         wmix1_sb[:, :], start=True, stop=True)
                    dst = h1t_all[:, i0:i0 + g, m * 128:(m + 1) * 128]
                    if m == 0 and gi < 2:
                        # first groups of each batch on ACT: the DVE queue
                        # still holds the last h0 casts, and psB (2 bufs)
                        # would stall the PE ~0.8us waiting for these relus
                        nc.scalar.activation(dst, bp[:, 0:g, :], AF.Relu)
                    else:
                        nc.vector.tensor_relu(dst, bp[:, 0:g, :])
            h2_tiles = [h2_p.tile([120, N], BF16, name="h2", tag=f"h2_{m}",
                                  bufs=1) for m in range(4)]
            for m, (mo, ms) in enumerate(MB):
                for (co, cs) in CH800:
                    ps = psA.tile([128, 400], F32, name="h1_ps", tag="psA")
                    if HOP1_FP8:
                        for p in range(7):
                            nc.tensor.matmul(
                                ps[:, :],
                                h1t[p][:, :, m * 128:(m + 1) * 128],
                                adj1_rhs(p, co, cs),
                                start=(p == 0), stop=(p == 6), perf_mode=DR)
                    else:
                        for kt in range(NKT):
                            nc.tensor.matmul(
                                ps[:, :],
                                h1t[kt][:, m * 128:(m + 1) * 128],
                                adj_rhs(kt, 800 + co, cs),
                                start=(kt == 0), stop=(kt == NKT - 1))
                    nc.scalar.copy(h2_tiles[m][:, co:co + cs], ps[0:120, :])
            def mix2_one(m, co, cs):
                kk = 120 if m < 3 else 80
                ps = psA.tile([128, 400], F32, name="b2_ps", tag="psA")
                nc.tensor.matmul(ps[:, :], wmix2_sb[0:kk, :],
                                 h2_tiles[m][0:kk, co:co + cs],
                                 start=True, stop=True)
                # fused relu(mix2) + dres in one DVE op: (ps max 0) add dres
                nc.vector.scalar_tensor_tensor(
                    oraw_sb[m][:, co:co + cs], ps[0:120, :], 0.0,
                    dres_sb[m][:, co:co + cs],
                    op0=ALU.max, op1=ALU.add)

            if tail_hook is None:
                for m in range(4):
                    for (co, cs) in CH800:
                        mix2_one(m, co, cs)
            else:
                # last batch: finish each 400-wide column half across all m,
                # then immediately emit its skip+residual epilogue -- the
                # first half's epilogue overlaps the second half's mix2
                for (co, cs) in CH800:
                    for m in range(4):
                        mix2_one(m, co, cs)
                    tail_hook(co, cs)

        def skip_ch(oraw_sb, fs, co, cs):
            for sm in range(4):
                ps = psA.tile([128, 400], F32, name="sk_ps", tag="psA")
                for kt in range(4):
                    nc.tensor.matmul(
                        ps[:, :],
                        wskip_sb[:, kt, sm * 128:(sm + 1) * 128],
                        oraw_sb[kt][:, co:co + cs],
                        start=(kt == 0), stop=(kt == 3))
                if has_bskip:
                    nc.scalar.activation(fs[:, sm, co:co + cs], ps[0:120, :],
                                         AF.Identity,
                                         bias=bskip_sb[:, sm, :])
                else:
                    nc.scalar.copy(fs[:, sm, co:co + cs], ps[0:120, :])

        def skip_part(b, oraw_sb, fs):
            for (co, cs) in CH800:
                skip_ch(oraw_sb, fs, co, cs)

        def fin_add(fr, m, co, cs, oraw_sb, res_src):
            nc.vector.scalar_tensor_tensor(
                fr[:, m, co:co + cs], oraw_sb[m][:, co:co + cs], BN_SCALE,
                res_src, op0=ALU.mult, op1=ALU.add)
            if has_bres:
                nc.vector.tensor_scalar_add(fr[:, m, co:co + cs],
                                            fr[:, m, co:co + cs],
                                            bres_sb[:, :])

        def epilogue_b(b, xp_sb, oraw_sb, res3_sb=None):
            fs = fin_p.tile([120, 4, N], BF16, name="fsk", tag="fsk", bufs=1)
            skip_part(b, oraw_sb, fs)
            fr = fin_p.tile([120, 4, N], BF16, name="fre", tag="fre", bufs=1)
            for m in range(4):
                for (co, cs) in CH800:
                    if res3_sb is None:
                        ps = res_mm(xp_sb, m, co, cs)
                        fin_add(fr, m, co, cs, oraw_sb, ps[0:120, :])
                    else:
                        fin_add(fr, m, co, cs, oraw_sb,
                                res3_sb[:, m, co:co + cs])
            eng_o = nc.sync if b % 2 == 0 else nc.gpsimd
            if b == BL - 1:
                # split the final writes so the drain starts ASAP
                for m in range(4):
                    nc.sync.dma_start(out_d[b, 0, :, m, :], fr[:, m, :])
                    nc.gpsimd.dma_start(out_d[b, 1, :, m, :], fs[:, m, :])
            else:
                eng_o.dma_start(out_d[b, 0], fr[:])
                eng_o.dma_start(out_d[b, 1], fs[:])

        # ---- ramp: all of adj first (its only dep, nv, lands earliest);
        # tconv(0)'s inputs arrive while the 52 adj matmuls run ----
        emit_adj(CH1600)

        # ---- per-batch software pipeline ----
        prev = None
        res3_sb = None
        for b in range(BL):
            if b >= 2:
                xp_sb_all[b] = load_xp(b)
                wxt_sb_all[b] = load_wxt(b)
            dres_sb = tconv_b(b, xp_sb_all[b])
            if b == BL - 1:
                # hoist the last batch's residual conv so the tail is only
                # skip matmuls + the final DMA drain
                res3_sb = const.tile([120, 4, N], BF16, name="res3")
                for m in range(4):
                    for (co, cs) in CH800:
                        ps = res_mm(xp_sb_all[b], m, co, cs)
                        nc.scalar.copy(res3_sb[:, m, co:co + cs], ps[0:120, :])
            if prev is not None:
                epilogue_b(*prev)
            oraw_sb = [oraw_p.tile([120, N], BF16, name=f"oraw{m}",
                                   tag=f"oraw{b % 2}_{m}", bufs=1)
                       for m in range(4)]
            hook = None
            if b == BL - 1:
                # last batch: fold the epilogue into the hop1/mix2 column
                # sweep so only the second half's skip trails the final hop
                fr3 = fin_p.tile([120, 4, N], BF16, name="fre", tag="fre",
                                 bufs=1)
                fs3 = fin_p.tile([120, 4, N], BF16, name="fsk", tag="fsk",
                                 bufs=1)

                def hook(co, cs, b=b, oraw_sb=oraw_sb):
                    # all final writes on sync: gpsimd's end-of-program DGE
                    # drain runs FIRST in the teardown round-robin and would
                    # expose the last transfer (~6us); sync's runs last
                    skip_ch(oraw_sb, fs3, co, cs)
                    for m in range(4):
                        fin_add(fr3, m, co, cs, oraw_sb,
                                res3_sb[:, m, co:co + cs])
                    nc.sync.dma_start(out_d[b, 0, :, :, co:co + cs],
                                      fr3[:, :, co:co + cs])
                    nc.sync.dma_start(out_d[b, 1, :, :, co:co + cs],
                                      fs3[:, :, co:co + cs])
            hops_b(b, wxt_sb_all[b], dres_sb, oraw_sb, tail_hook=hook)
            prev = (b, xp_sb_all[b], oraw_sb)


_CACHE = {}


def kernel(**inputs):
    w = _prep_weights(inputs)
    xp_cores, wxt_cores = _prep_data(inputs)

    key = ("prog", w['has_bres'], w['has_bskip'], HOP1_FP8)
    if key not in _CACHE:
        _CACHE[key] = _build_program(has_bres=w['has_bres'],
                                     has_bskip=w['has_bskip'])
    nc = _CACHE[key]

    in_maps = []
    for core in range(NCORES):
        in_maps.append({
            "xp": xp_cores[core],
            "wxt": wxt_cores[core],
            "nv1t": w['nv1t'],
            "nv2": w['nv2'],
            "wbig": w['wbig'],
            "wmix1": w['wmix1'],
            "wmix2": w['wmix2'],
            "wskip": w['wskip'],
            "wres": w['wres'],
            "bias_f": w['bias_f'],
            "bias_g": w['bias_g'],
            "bskip": w['bskip_tile'],
            "bres": w['bres_tile'],
        })

    import os
    trace = bool(int(os.environ.get("KERNEL_TRACE", "0")))
    res = run_bass_kernel_spmd(nc, in_maps, core_ids=list(range(NCORES)),
                               trace=trace)
    kernel.last_result = res
    outs = [r["out"] for r in res.results]        # each (BL, 2, 120, 4, 800) bf16
    full = np.concatenate(outs, axis=0).astype(np.float32)   # (32, 2, 120, 4, 800)
    full = full.transpose(0, 1, 3, 2, 4)                     # (32, 2, 4, 120, 800)
    res_part = full[:, 0].reshape(B, 480, N)[:, :RQ]         # (B, 440, 800)
    skip_part = full[:, 1].reshape(B, 480, N)                # (B, 480, 800)
    out = np.empty((B, C, R + 12, N), np.float32)
    out[:, :, :R] = res_part.reshape(B, R, C, N).transpose(0, 2, 1, 3)
    out[:, :, R:] = skip_part.reshape(B, 12, C, N).transpose(0, 2, 1, 3)
    return np.ascontiguousarray(out)


# revision 80
# speedup vs baseline: 1.0034x; 1.0034x over previous
"""Trainium2 Bass kernel for nn_Net_66975720014255 (gnn_message_passing).

Sharding: data-parallel over batch B=32 across 8 NeuronCores (4 batches per
core); adjacency and all weights replicated. No collectives.

v2 design (per core, C=40, T=12, N=800, R=11):
  - everything bf16 (PSUM accumulation f32); output shipped bf16, host upcasts
  - weight/window matrices column-padded so every matmul has M=128 (FWL)
  - node dim K padded 1600 -> 1664 = 13x128 uniform K-tiles
  - adjacency computed ON DEVICE from nv1/nv2 (0.26MB DMA instead of 5.1MB,
    and the K=40 matmuls warm the PE during the input-DMA ramp)
  - optional hop1 in fp8-e4m3 DoubleRow (2x K-rate): adj upper half + h1t are
    cast to fp8 on device; numerically validated (rel err ~4e-3 total)
  - warm-up matmuls at t=0 keep HAM at full clock through the DMA ramp
  - few, large DMAs on the idle sync/gpsimd queues; last batch's residual
    matmuls hoisted before the hops so the tail is only skip + out-DMA drain
"""

import sys

if '/opt/trn_rl_repo' not in sys.path:
    sys.path.insert(0, '/opt/trn_rl_repo')

import numpy as np
import ml_dtypes

import concourse.bass as bass  # noqa: F401
import concourse.tile as tile
from concourse import bacc, mybir
from concourse.bass_utils import run_bass_kernel_spmd

# ----- problem constants (hardcoded per contract) -----
B, C, T, N = 32, 40, 12, 800
R = T - 1                    # 11
N2 = 2 * N                   # 1600
NCORES = 8
BL = B // NCORES             # 4 local batches per core
BN_SCALE = float(1.0 / np.sqrt(1.0 + 1e-5))

Q = T * C                    # 480 rows (t,c) per batch
RQ = R * C                   # 440 rows (r,c) per batch
NKT = 13                     # node K-tiles: 13*128 = 1664 (1600 padded)
N2P = NKT * 128              # 1664
MB = [(0, 120), (120, 120), (240, 120), (360, 80)]   # (r,c) row blocks (real)
CH800 = [(0, 400), (400, 400)]
CH1600 = [(0, 400), (400, 400), (800, 400), (1200, 400)]

F32 = mybir.dt.float32
BF16 = mybir.dt.bfloat16
F8E4 = mybir.dt.float8e4

HOP1_FP8 = True              # hop1 via fp8 DoubleRow (2x K-rate)
ADJ_FP8 = True               # single fp8 adjacency: hop0 rhs fp8 (lhsT bf16)
WARMUP_MM = 24

_bf = ml_dtypes.bfloat16
_f8 = ml_dtypes.float8_e4m3


# ---------------------------------------------------------------------------
# host-side preparation (pure numpy)
# ---------------------------------------------------------------------------

def _pad512(Wb):
    """[rows, 440-or-480] -> [rows, 512]: each 120-col block padded to 128
    (440-col input: 4th block is 80 wide)."""
    rows, cols = Wb.shape
    out = np.zeros((rows, 512), np.float32)
    for m in range(4):
        w = min(120, cols - m * 120)
        out[:, m * 128:m * 128 + w] = Wb[:, m * 120:m * 120 + w]
    return out


def _ktile4(Wp, dt=None, part=120):
    """[rows<=480, 512] -> [part, 4, 512]: K-tiles of 120 real rows,
    zero-padded to `part` partitions."""
    full = np.zeros((480, 512), np.float32)
    full[:Wp.shape[0]] = Wp
    out = np.zeros((part, 4, 512), np.float32)
    out[:120] = full.reshape(4, 120, 512).transpose(1, 0, 2)
    return np.ascontiguousarray(out.astype(dt or _bf))


def _prep_weights(inp):
    f32 = np.float32
    nv1 = np.asarray(inp['nv1'], f32)          # (1600, 40)
    nv2 = np.asarray(inp['nv2'], f32)          # (40, 1600)
    nv1t = np.zeros((40, N2P), f32)
    nv1t[:, :N2] = nv1.T
    nv1t = np.ascontiguousarray(nv1t.astype(_bf))
    nv2b = np.ascontiguousarray(nv2.astype(_bf))

    def wbig(W):
        Wb = np.zeros((Q, RQ), f32)
        W0, W1 = np.asarray(W[:, :, 0], f32), np.asarray(W[:, :, 1], f32)
        for r in range(R):
            Wb[r * C:(r + 1) * C, r * C:(r + 1) * C] = W0.T
            Wb[(r + 1) * C:(r + 2) * C, r * C:(r + 1) * C] = W1.T
        return Wb

    wbig2 = np.ascontiguousarray(
        np.stack([_ktile4(_pad512(wbig(np.asarray(inp['W_f']))), _f8, 128),
                  _ktile4(_pad512(wbig(np.asarray(inp['W_g']))), _f8, 128)]
                 ).transpose(1, 0, 2, 3))          # [128, 2, 4, 512] fp8

    def blkdiag3(A, cols=120):                 # A (40, 40) -> [120, cols]
        M = np.zeros((120, cols), f32)
        for j in range(3):
            M[j * C:(j + 1) * C, j * C:(j + 1) * C] = A
        return M

    wmix1 = np.ascontiguousarray(
        blkdiag3(np.asarray(inp['W_gcn'][0], f32).T, 128).astype(_bf))
    wmix2 = np.ascontiguousarray(
        blkdiag3(np.asarray(inp['W_gcn'][1], f32).T, 128).astype(_bf))

    eye = np.eye(C, dtype=f32)
    Ws = np.asarray(inp['W_skip'], f32) * BN_SCALE            # (12, 11)
    wskip = np.zeros((RQ, 480), f32)
    for s in range(12):
        for r in range(R):
            wskip[r * C:(r + 1) * C, s * C:(s + 1) * C] = Ws[s, r] * eye
    wskip = _ktile4(_pad512(wskip))

    wres = np.zeros((Q, RQ), f32)
    Wr = np.asarray(inp['W_res'], f32) * BN_SCALE             # (11, 12)
    for t in range(T):
        for r in range(R):
            wres[t * C:(t + 1) * C, r * C:(r + 1) * C] = Wr[r, t] * eye
    wres = _ktile4(_pad512(wres), _f8, 128)

    bias_f = np.ascontiguousarray(np.tile(np.asarray(inp['b_f'], f32), 3)[:, None])
    bias_g = np.ascontiguousarray(np.tile(np.asarray(inp['b_g'], f32), 3)[:, None])

    bs = np.asarray(inp['b_skip'], f32) * BN_SCALE
    bskip_tile = np.zeros((120, 4, 1), f32)
    for sm in range(4):
        for p in range(120):
            bskip_tile[p, sm, 0] = bs[sm * 3 + p // C]
    bres = np.asarray(inp['b_res'], f32) * BN_SCALE
    bres_tile = np.zeros((120, 1), f32)
    for p in range(120):
        r = p // C
        bres_tile[p, 0] = bres[r] if r < R else 0.0

    return dict(nv1t=nv1t, nv2=nv2b, wbig=wbig2, wmix1=wmix1, wmix2=wmix2,
                wskip=wskip, wres=wres, bias_f=bias_f, bias_g=bias_g,
                bskip_tile=np.ascontiguousarray(bskip_tile),
                has_bskip=bool(np.any(bs)),
                bres_tile=bres_tile, has_bres=bool(np.any(bres)))


def _prep_data(inp):
    f32 = np.float32
    x = np.asarray(inp['x'], f32) + np.asarray(inp['t_emb'], f32) \
        + np.asarray(inp['s_emb'], f32)                        # (B,C,T,N)
    xp = np.ascontiguousarray(x.transpose(0, 2, 1, 3)).reshape(B, Q, N)
    # xp tiles (fp8 for tconv/res DoubleRow): [B, 128, 4, 800]
    xp_t = np.zeros((B, 128, 4, N), np.float32)
    xp_t[:, :120] = xp.reshape(B, 4, 120, N).transpose(0, 2, 1, 3)
    xp_t = np.ascontiguousarray(xp_t.astype(_f8))
    # windowed transpose, rows k in [0,1600): k<800 -> x'T[:, :440];
    # k>=800 -> x'T[:, 40:480]; cols padded per 120->128 block, rows ->1664
    xpt = np.ascontiguousarray(x.transpose(0, 3, 2, 1)).reshape(B, N, Q)
    wxt = np.zeros((B, N2P, 512), f32)
    for m in range(4):
        w = 120 if m < 3 else 80
        lo, hi = m * 120, m * 120 + w
        wxt[:, :N, m * 128:m * 128 + w] = xpt[:, :, lo:hi]
        wxt[:, N:N2, m * 128:m * 128 + w] = xpt[:, :, C + lo:C + hi]
    wxt = np.ascontiguousarray(
        wxt.reshape(B, NKT, 128, 512).transpose(0, 2, 1, 3).astype(_bf))
    xp_cores = [np.ascontiguousarray(xp_t[i * BL:(i + 1) * BL])
                for i in range(NCORES)]
    wxt_cores = [np.ascontiguousarray(wxt[i * BL:(i + 1) * BL])
                 for i in range(NCORES)]
    return xp_cores, wxt_cores


# ---------------------------------------------------------------------------
# device program
# ---------------------------------------------------------------------------

def _build_program(has_bres, has_bskip):
    nc = bacc.Bacc("TRN2", target_bir_lowering=False, debug=False,
                   enable_asserts=False, num_devices=NCORES)

    xp_d = nc.dram_tensor("xp", [BL, 128, 4, N], F8E4, kind="ExternalInput").ap()
    wxt_d = nc.dram_tensor("wxt", [BL, 128, NKT, 512], BF16,
                           kind="ExternalInput").ap()
    nv1t_d = nc.dram_tensor("nv1t", [40, N2P], BF16, kind="ExternalInput").ap()
    nv2_d = nc.dram_tensor("nv2", [40, N2], BF16, kind="ExternalInput").ap()
    wbig_d = nc.dram_tensor("wbig", [128, 2, 4, 512], F8E4,
                            kind="ExternalInput").ap()
    wmix1_d = nc.dram_tensor("wmix1", [120, 128], BF16, kind="ExternalInput").ap()
    wmix2_d = nc.dram_tensor("wmix2", [120, 128], BF16, kind="ExternalInput").ap()
    wskip_d = nc.dram_tensor("wskip", [120, 4, 512], BF16,
                             kind="ExternalInput").ap()
    wres_d = nc.dram_tensor("wres", [128, 4, 512], F8E4,
                            kind="ExternalInput").ap()
    biasf_d = nc.dram_tensor("bias_f", [120, 1], F32, kind="ExternalInput").ap()
    biasg_d = nc.dram_tensor("bias_g", [120, 1], F32, kind="ExternalInput").ap()
    bskip_d = nc.dram_tensor("bskip", [120, 4, 1], F32, kind="ExternalInput").ap()
    bres_d = nc.dram_tensor("bres", [120, 1], F32, kind="ExternalInput").ap()
    # out[b, 0, p, m, n] = final rows (r,c)=m*120+p (m=3: p<80 real)
    # out[b, 1, p, m, n] = skip rows (s,c)=m*120+p
    out_d = nc.dram_tensor("out", [BL, 2, 120, 4, N], BF16,
                           kind="ExternalOutput").ap()

    with tile.TileContext(nc) as tc:
        _emit(nc, tc, xp_d, wxt_d, nv1t_d, nv2_d, wbig_d, wmix1_d, wmix2_d,
              wskip_d, wres_d, biasf_d, biasg_d, bskip_d, bres_d, out_d,
              has_bres, has_bskip)
    nc.compile()
    return nc


def _emit(nc, tc, xp_d, wxt_d, nv1t_d, nv2_d, wbig_d, wmix1_d, wmix2_d,
          wskip_d, wres_d, biasf_d, biasg_d, bskip_d, bres_d, out_d,
          has_bres, has_bskip):
    from contextlib import ExitStack
    AF = mybir.ActivationFunctionType
    ALU = mybir.AluOpType
    DR = mybir.MatmulPerfMode.DoubleRow
    ctx = ExitStack()
    with ctx:
        const = ctx.enter_context(tc.tile_pool(name="const", bufs=1))
        xp_p = ctx.enter_context(tc.tile_pool(name="xp", bufs=2))
        xpt_p = ctx.enter_context(tc.tile_pool(name="xpt", bufs=2))
        dres_p = ctx.enter_context(tc.tile_pool(name="dres", bufs=1))
        hop0_p = ctx.enter_context(tc.tile_pool(name="hop0sb", bufs=1))
        h1t_p = ctx.enter_context(tc.tile_pool(name="h1t", bufs=1))
        h2_p = ctx.enter_context(tc.tile_pool(name="h2sb", bufs=1))
        oraw_p = ctx.enter_context(tc.tile_pool(name="oraw", bufs=1))
        tmp_p = ctx.enter_context(tc.tile_pool(name="tmp", bufs=2))
        fin_p = ctx.enter_context(tc.tile_pool(name="fin", bufs=2))
        psA = ctx.enter_context(tc.tile_pool(name="psA", bufs=6, space="PSUM"))
        psB = ctx.enter_context(tc.tile_pool(name="psB", bufs=2, space="PSUM"))

        # ---- warm-up: PE busy from t=0 so HAM reaches 2.4GHz early ----
        dummy = const.tile([128, 400], BF16, name="dummy")
        nc.vector.memset(dummy[:], 0.125)
        def warm_mm():
            ps = psB.tile([128, 400], F32, name="warm", tag="psB")
            nc.tensor.matmul(ps[:, :], dummy[:, 0:128], dummy[:, :],
                             start=True, stop=True)

        for i in range(WARMUP_MM):
            warm_mm()

        # ---- DMA: nv first (adj compute), xp+wbig (tconv), wxt per batch ----
        # nv first on two queues: adj compute is the PE's first real work and
        # the DMA rings take ~5us to produce their first bytes
        nv1t_sb = const.tile([40, N2P], BF16, name="nv1t")
        nc.sync.dma_start(nv1t_sb[:], nv1t_d[:])
        nv2_sb = const.tile([40, N2], BF16, name="nv2")
        nc.scalar.dma_start(nv2_sb[:], nv2_d[:])
        biasf_sb = const.tile([120, 1], F32, name="biasf")
        nc.sync.dma_start(biasf_sb[:], biasf_d[:])
        biasg_sb = const.tile([120, 1], F32, name="biasg")
        nc.sync.dma_start(biasg_sb[:], biasg_d[:])

        def load_xp(b):
            t = xp_p.tile([128, 4, N], F8E4, name="xp", tag="xp", bufs=2)
            nc.sync.dma_start(t[:], xp_d[b])
            return t

        xp0 = load_xp(0)
        # wbig on the scalar queue: issues at t=0 in parallel with xp0 (sync)
        # and nv/wxt (gpsimd) so tconv's inputs all land ~7us earlier
        wbig_sb = const.tile([128, 2, 4, 512], F8E4, name="wbig")
        nc.scalar.dma_start(wbig_sb[:], wbig_d[:])

        def load_wxt(b):
            t = xpt_p.tile([128, NKT, 512], BF16, name="wxt", tag="wxt", bufs=2)
            nc.gpsimd.dma_start(t[:], wxt_d[b])
            return t

        wxt0 = load_wxt(0)

        # remaining weights (needed from mix1 / epilogue onward)
        wmix1_sb = const.tile([120, 128], BF16, name="wmix1")
        nc.sync.dma_start(wmix1_sb[:], wmix1_d[:])
        wmix2_sb = const.tile([120, 128], BF16, name="wmix2")
        nc.sync.dma_start(wmix2_sb[:], wmix2_d[:])
        wres_sb = const.tile([128, 4, 512], F8E4, name="wres")
        nc.sync.dma_start(wres_sb[:], wres_d[:])
        wskip_sb = const.tile([120, 4, 512], BF16, name="wskip")
        nc.sync.dma_start(wskip_sb[:], wskip_d[:])
        bres_sb = const.tile([120, 1], F32, name="bres_t")
        nc.sync.dma_start(bres_sb[:], bres_d[:])
        if has_bskip:
            bskip_sb = const.tile([120, 4, 1], F32, name="bskip_t")
            nc.sync.dma_start(bskip_sb[:], bskip_d[:])

        # batches 0/1 prefetched here; 2/3 loaded inside the loop so the
        # tag-based buffer reuse sees its prior readers (WAR ordering)
        xp_sb_all = [xp0, load_xp(1), None, None]
        wxt_sb_all = [wxt0, load_wxt(1), None, None]

        # ---- adjacency on device: adj = relu(nv1 @ nv2) ----
        if ADJ_FP8:
            # single fp8 copy in DoubleRow-paired layout [128, ksub=2, 1600]:
            # hop0 reads it as a plain (mixed-dtype) rhs, hop1 as DR pairs
            adj8p = [const.tile([128, 2, N2], F8E4, name=f"adj8p{p}")
                     for p in range(7)]
            nc.vector.memset(adj8p[6][:, 1, :], 0.0)

            def adj_rhs(kt, co, cs):
                return adj8p[kt // 2][:, kt % 2, co:co + cs]

            def adj1_rhs(p, co, cs):
                return adj8p[p][:, :, 800 + co:800 + co + cs]
        else:
            adj_sb = [const.tile([128, N2], BF16, name=f"adj{i}")
                      for i in range(NKT)]

            def adj_rhs(kt, co, cs):
                return adj_sb[kt][:, co:co + cs]

            if HOP1_FP8:
                # paired fp8 copy of adj[:, 800:1600]: [128, ksub=2, 800] x 7
                adj8_sb = [const.tile([128, 2, N], F8E4, name=f"adj8_{p}")
                           for p in range(7)]
                nc.vector.memset(adj8_sb[6][:, 1, :], 0.0)

                def adj1_rhs(p, co, cs):
                    return adj8_sb[p][:, :, co:co + cs]   # k-tile 13 = zero pad
        def emit_adj(chunks):
            for (co, cs) in chunks:
                for i in range(NKT):
                    ps = psA.tile([128, 400], F32, name="adj_ps", tag="psA")
                    nc.tensor.matmul(ps[:, :], nv1t_sb[:, i * 128:(i + 1) * 128],
                                     nv2_sb[:, co:co + cs], start=True, stop=True)
                    # split the casts over ACT/DVE so neither queue alone
                    # paces the PSUM-bank recycling
                    eng = nc.scalar if i % 2 == 0 else nc.vector
                    if ADJ_FP8:
                        dst = adj8p[i // 2][:, i % 2, co:co + cs]
                    else:
                        dst = adj_sb[i][:, co:co + cs]
                    if eng is nc.scalar:
                        eng.activation(dst, ps[:, :], AF.Relu)
                    else:
                        eng.tensor_relu(dst, ps[:, :])
                    if not ADJ_FP8 and HOP1_FP8 and co >= 800:
                        eng2 = nc.vector if i % 2 == 0 else nc.scalar
                        if eng2 is nc.scalar:
                            eng2.activation(
                                adj8_sb[i // 2][:, i % 2,
                                                co - 800:co - 800 + cs],
                                ps[:, :], AF.Relu)
                        else:
                            eng2.tensor_relu(
                                adj8_sb[i // 2][:, i % 2,
                                                co - 800:co - 800 + cs],
                                ps[:, :])

        def tconv_b(b, xp_sb, dres_sb=None):
            if dres_sb is None:
                dres_sb = [dres_p.tile([120, N], BF16, name=f"dres{m}",
                                       tag=f"dres{m}", bufs=1) for m in range(4)]
            for m, (mo, ms) in enumerate(MB):
                gate_sb = {}
                for g, bias_sb in ((0, biasf_sb), (1, biasg_sb)):
                    gt = tmp_p.tile([120, N], BF16, name="gate",
                                    tag=f"gate{g}", bufs=2)
                    gate_sb[g] = gt
                    for (co, cs) in CH800:
                        ps = psA.tile([128, 400], F32, name="tc_ps", tag="psA")
                        if m < 3:
                            nc.tensor.matmul(
                                ps[:, :],
                                wbig_sb[:, g, m:m + 2, m * 128:(m + 1) * 128],
                                xp_sb[:, m:m + 2, co:co + cs],
                                start=True, stop=True, perf_mode=DR)
                        else:
                            nc.tensor.matmul(
                                ps[:, :],
                                wbig_sb[:, g, 3, m * 128:(m + 1) * 128],
                                xp_sb[:, 3, co:co + cs],
                                start=True, stop=True)
                        nc.scalar.activation(
                            gt[:, co:co + cs], ps[0:120, :],
                            AF.Tanh if g == 0 else AF.Sigmoid,
                            bias=bias_sb[:, :])
                nc.vector.tensor_mul(dres_sb[m][:, :],
                                     gate_sb[0][:, :], gate_sb[1][:, :])
            return dres_sb

        def res_mm(xp_sb, m, co, cs):
            ps = psA.tile([128, 400], F32, name="rs_ps", tag="psA")
            for j in range(2):
                nc.tensor.matmul(ps[:, :],
                                 wres_sb[:, 2 * j:2 * j + 2,
                                         m * 128:(m + 1) * 128],
                                 xp_sb[:, 2 * j:2 * j + 2, co:co + cs],
                                 start=(j == 0), stop=(j == 1), perf_mode=DR)
            return ps

        def hops_b(b, wxt_sb, dres_sb, oraw_sb, tail_hook=None):
            # hop0 + mix1 -> h1t ; hop1 + mix2 (+dres add) -> oraw
            h0_tiles = [hop0_p.tile([128, N2P], BF16, name="h0", tag=f"h0_{m}",
                                    bufs=1) for m in range(4)]
            # batch 0: ch-outer (each chunk only needs 13 adj casts, so hop0
            # starts before the full cast sweep). later batches: m-outer, so
            # each h0 tile's casts finish early and mix1's relus never wait.
            if b == 0:
                order = [(co, cs, m) for (co, cs) in CH1600 for m in range(4)]
            else:
                order = [(co, cs, m) for m in range(4) for (co, cs) in CH1600]
            for (co, cs, m) in order:
                ps = psA.tile([128, 400], F32, name="h0_ps", tag="psA")
                for kt in range(NKT):
                    nc.tensor.matmul(
                        ps[:, :],
                        wxt_sb[:, kt, m * 128:(m + 1) * 128],
                        adj_rhs(kt, co, cs),
                        start=(kt == 0), stop=(kt == NKT - 1))
                nc.vector.tensor_copy(h0_tiles[m][:, co:co + cs], ps[:, :])
            for m in range(4):
                nc.vector.memset(h0_tiles[m][:, N2:N2P], 0.0)
            if HOP1_FP8:
                # one tile [128, 14 ksub, 512]: slices [:, 2p:2p+2, mslice]
                # feed hop1's DoubleRow pairs directly
                h1t_all = h1t_p.tile([128, 14, 512], F8E4, name="h1t",
                                     tag="h1t", bufs=1)
                h1t = [h1t_all[:, 2 * p:2 * p + 2, :] for p in range(7)]
                nc.vector.memset(h1t_all[:, 13, :], 0.0)
            else:
                h1t_all = h1t_p.tile([128, NKT, 512], BF16, name="h1t",
                                     tag="h1t", bufs=1)
                h1t = [h1t_all[:, kt, :] for kt in range(NKT)]

            for m in range(4):
                h0 = h0_tiles[m]
                # 4 mix1 matmuls per PSUM bank, ONE wide relu per group:
                # amortizes the DVE per-op overhead so it outruns the PE
                for gi, i0 in enumerate(range(0, NKT, 4)):
                    g = min(4, NKT - i0)
                    bp = psB.tile([128, 4, 128], F32, name="b1_ps", tag="psB")
                    for j in range(g):
                        i = i0 + j
                        nc.tensor.matmul(bp[:, j, :],
                                         h0[0:120, i * 128:(i + 1) * 128],
                                         wmix1_sb[:, :], start=True, stop=True)
                    dst = h1t_all[:, i0:i0 + g, m * 128:(m + 1) * 128]
                    if m == 0 and gi < 2:
                        # first groups of each batch on ACT: the DVE queue
                        # still holds the last h0 casts, and psB (2 bufs)
                        # would stall the PE ~0.8us waiting for these relus
                        nc.scalar.activation(dst, bp[:, 0:g, :], AF.Relu)
                    else:
                        nc.vector.tensor_relu(dst, bp[:, 0:g, :])
            h2_tiles = [h2_p.tile([120, N], BF16, name="h2", tag=f"h2_{m}",
                                  bufs=1) for m in range(4)]
            for m, (mo, ms) in enumerate(MB):
                for (co, cs) in CH800:
                    ps = psA.tile([128, 400], F32, name="h1_ps", tag="psA")
                    if HOP1_FP8:
                        for p in range(7):
                            nc.tensor.matmul(
                                ps[:, :],
                                h1t[p][:, :, m * 128:(m + 1) * 128],
                                adj1_rhs(p, co, cs),
                                start=(p == 0), stop=(p == 6), perf_mode=DR)
                    else:
                        for kt in range(NKT):
                            nc.tensor.matmul(
                                ps[:, :],
                                h1t[kt][:, m * 128:(m + 1) * 128],
                                adj_rhs(kt, 800 + co, cs),
                                start=(kt == 0), stop=(kt == NKT - 1))
                    nc.scalar.copy(h2_tiles[m][:, co:co + cs], ps[0:120, :])
            def mix2_one(m, co, cs):
                kk = 120 if m < 3 else 80
                ps = psA.tile([128, 400], F32, name="b2_ps", tag="psA")
                nc.tensor.matmul(ps[:, :], wmix2_sb[0:kk, :],
                                 h2_tiles[m][0:kk, co:co + cs],
                                 start=True, stop=True)
                # fused relu(mix2) + dres in one DVE op: (ps max 0) add dres
                nc.vector.scalar_tensor_tensor(
                    oraw_sb[m][:, co:co + cs], ps[0:120, :], 0.0,
                    dres_sb[m][:, co:co + cs],
                    op0=ALU.max, op1=ALU.add)

            if tail_hook is None:
                for m in range(4):
                    for (co, cs) in CH800:
                        mix2_one(m, co, cs)
            else:
                # last batch: finish each 400-wide column half across all m,
                # then immediately emit its skip+residual epilogue -- the
                # first half's epilogue overlaps the second half's mix2
                for (co, cs) in CH800:
                    for m in range(4):
                        mix2_one(m, co, cs)
                    tail_hook(co, cs)

        def skip_ch(oraw_sb, fs, co, cs):
            for sm in range(4):
                ps = psA.tile([128, 400], F32, name="sk_ps", tag="psA")
                for kt in range(4):
                    nc.tensor.matmul(
                        ps[:, :],
                        wskip_sb[:, kt, sm * 128:(sm + 1) * 128],
                        oraw_sb[kt][:, co:co + cs],
                        start=(kt == 0), stop=(kt == 3))
                if has_bskip:
                    nc.scalar.activation(fs[:, sm, co:co + cs], ps[0:120, :],
                                         AF.Identity,
                                         bias=bskip_sb[:, sm, :])
                else:
                    nc.scalar.copy(fs[:, sm, co:co + cs], ps[0:120, :])

        def skip_part(b, oraw_sb, fs):
            for (co, cs) in CH800:
                skip_ch(oraw_sb, fs, co, cs)

        def fin_add(fr, m, co, cs, oraw_sb, res_src):
            nc.vector.scalar_tensor_tensor(
                fr[:, m, co:co + cs], oraw_sb[m][:, co:co + cs], BN_SCALE,
                res_src, op0=ALU.mult, op1=ALU.add)
            if has_bres:
                nc.vector.tensor_scalar_add(fr[:, m, co:co + cs],
                                            fr[:, m, co:co + cs],
                                            bres_sb[:, :])

        def epilogue_b(b, xp_sb, oraw_sb, res3_sb=None):
            fs = fin_p.tile([120, 4, N], BF16, name="fsk", tag="fsk", bufs=1)
            skip_part(b, oraw_sb, fs)
            fr = fin_p.tile([120, 4, N], BF16, name="fre", tag="fre", bufs=1)
            for m in range(4):
                for (co, cs) in CH800:
                    if res3_sb is None:
                        ps = res_mm(xp_sb, m, co, cs)
                        fin_add(fr, m, co, cs, oraw_sb, ps[0:120, :])
                    else:
                        fin_add(fr, m, co, cs, oraw_sb,
                                res3_sb[:, m, co:co + cs])
            eng_o = nc.sync if b % 2 == 0 else nc.gpsimd
            if b == BL - 1:
                # split the final writes so the drain starts ASAP
                for m in range(4):
                    nc.sync.dma_start(out_d[b, 0, :, m, :], fr[:, m, :])
                    nc.gpsimd.dma_start(out_d[b, 1, :, m, :], fs[:, m, :])
            else:
                eng_o.dma_start(out_d[b, 0], fr[:])
                eng_o.dma_start(out_d[b, 1], fs[:])

        # ---- ramp: all of adj first (its only dep, nv, lands earliest);
        # tconv(0)'s inputs arrive while the 52 adj matmuls run ----
        emit_adj(CH1600)

        # ---- per-batch software pipeline ----
        prev = None
        res3_sb = None
        for b in range(BL):
            if b >= 2:
                xp_sb_all[b] = load_xp(b)
                wxt_sb_all[b] = load_wxt(b)
            dres_sb = tconv_b(b, xp_sb_all[b])
            if b == BL - 1:
                # hoist the last batch's residual conv so the tail is only
                # skip matmuls + the final DMA drain
                res3_sb = const.tile([120, 4, N], BF16, name="res3")
                for m in range(4):
                    for (co, cs) in CH800:
                        ps = res_mm(xp_sb_all[b], m, co, cs)
                        nc.scalar.copy(res3_sb[:, m, co:co + cs], ps[0:120, :])
            if prev is not None:
                epilogue_b(*prev)
            oraw_sb = [oraw_p.tile([120, N], BF16, name=f"oraw{m}",
                                   tag=f"oraw{b % 2}_{m}", bufs=1)
                       for m in range(4)]
            hook = None
            if b == BL - 1:
                # last batch: fold the epilogue into the hop1/mix2 column
                # sweep so only the second half's skip trails the final hop
                fr3 = fin_p.tile([120, 4, N], BF16, name="fre", tag="fre",
                                 bufs=1)
                fs3 = fin_p.tile([120, 4, N], BF16, name="fsk", tag="fsk",
                                 bufs=1)

                def hook(co, cs, b=b, oraw_sb=oraw_sb):
                    # all final writes on sync: gpsimd's end-of-program DGE
                    # drain runs FIRST in the teardown round-robin and would
                    # expose the last transfer (~6us); sync's runs last
                    skip_ch(oraw_sb, fs3, co, cs)
                    for m in range(4):
                        fin_add(fr3, m, co, cs, oraw_sb,
                                res3_sb[:, m, co:co + cs])
                    nc.sync.dma_start(out_d[b, 0, :, :, co:co + cs],
                                      fr3[:, :, co:co + cs])
                    nc.sync.dma_start(out_d[b, 1, :, :, co:co + cs],
                                      fs3[:, :, co:co + cs])
            hops_b(b, wxt_sb_all[b], dres_sb, oraw_sb, tail_hook=hook)
            prev = (b, xp_sb_all[b], oraw_sb)


_CACHE = {}


def kernel(**inputs):
    w = _prep_weights(inputs)
    xp_cores, wxt_cores = _prep_data(inputs)

    key = ("prog", w['has_bres'], w['has_bskip'], HOP1_FP8)
    if key not in _CACHE:
        _CACHE[key] = _build_program(has_bres=w['has_bres'],
                                     has_bskip=w['has_bskip'])
    nc = _CACHE[key]

    in_maps = []
    for core in range(NCORES):
        in_maps.append({
            "xp": xp_cores[core],
            "wxt": wxt_cores[core],
            "nv1t": w['nv1t'],
            "nv2": w['nv2'],
            "wbig": w['wbig'],
            "wmix1": w['wmix1'],
            "wmix2": w['wmix2'],
            "wskip": w['wskip'],
            "wres": w['wres'],
            "bias_f": w['bias_f'],
            "bias_g": w['bias_g'],
            "bskip": w['bskip_tile'],
            "bres": w['bres_tile'],
        })

    import os
    trace = bool(int(os.environ.get("KERNEL_TRACE", "0")))
    res = run_bass_kernel_spmd(nc, in_maps, core_ids=list(range(NCORES)),
                               trace=trace)
    kernel.last_result = res
    outs = [r["out"] for r in res.results]        # each (BL, 2, 120, 4, 800) bf16
    full = np.concatenate(outs, axis=0).astype(np.float32)   # (32, 2, 120, 4, 800)
    full = full.transpose(0, 1, 3, 2, 4)                     # (32, 2, 4, 120, 800)
    res_part = full[:, 0].reshape(B, 480, N)[:, :RQ]         # (B, 440, 800)
    skip_part = full[:, 1].reshape(B, 480, N)                # (B, 480, 800)
    out = np.empty((B, C, R + 12, N), np.float32)
    out[:, :, :R] = res_part.reshape(B, R, C, N).transpose(0, 2, 1, 3)
    out[:, :, R:] = skip_part.reshape(B, 12, C, N).transpose(0, 2, 1, 3)
    return np.ascontiguousarray(out)
